# revision 31
# baseline (speedup 1.0000x reference)
"""Deformable 3x3 conv (AdaptiveConv, N=16 C=256 H=W=32) on 8 trn2 cores. V2.

Data-parallel over batch: 2 samples per core. Changes vs V1:
  - gather table (4-corner bf16 pack per entry) built host-side; no
    on-device xdup build, gathers start as soon as indices are ready.
  - bilinear lerp folded into the PE: per corner px, matmul
    g_px^T @ diag(w_px) accumulated in PSUM replaces the DVE lerp and
    the separate PE transposes. DVE only builds diag blocks (4 small
    broadcast ops per tap).
"""
from contextlib import ExitStack

import numpy as np

try:
    import ml_dtypes
    _BF16 = ml_dtypes.bfloat16
except ImportError:  # pragma: no cover
    _BF16 = None

N, C_IN, C_OUT, H, W = 16, 256, 256, 32, 32
K = 9
HW = H * W
NCORES = 8
SPC = N // NCORES
NQROW = 1056  # gather table rows (>= 1025, padded)

_cache = {}


def _build():
    import concourse.bass as bass
    import concourse.mybir as mybir
    import concourse.tile as tile
    from concourse import bacc

    bf = mybir.dt.bfloat16
    f32 = mybir.dt.float32
    i8 = mybir.dt.int8
    i16 = mybir.dt.int16
    AOT = mybir.AluOpType

    nc = bacc.Bacc("TRN2", target_bir_lowering=False, debug=False,
                   num_devices=NCORES, num_swdge_queues=2)

    xq = nc.declare_dram_parameter("xq", [SPC, NQROW, 1024], bf,
                                   isOutput=False)
    off_w = nc.declare_dram_parameter("off_w", [SPC, 128, 8, 2 * K], f32,
                                      isOutput=False)
    base_w = nc.declare_dram_parameter("base_w", [128, 8, 2 * K], f32,
                                       isOutput=False)
    wt = nc.declare_dram_parameter("wt", [2 * K, 128, C_OUT], bf,
                                   isOutput=False)
    out_d = nc.declare_dram_parameter("out", [SPC, C_OUT, HW], bf,
                                      isOutput=True)

    istage = nc.dram_tensor("istage", [SPC, HW, K], i16)

    with tile.TileContext(nc) as tc, ExitStack() as ctx:
        cpool = ctx.enter_context(tc.tile_pool(name="const", bufs=1))
        ppool = ctx.enter_context(tc.tile_pool(name="pipe", bufs=2))
        tpool = ctx.enter_context(tc.tile_pool(name="tmp", bufs=2))
        gpool = ctx.enter_context(tc.tile_pool(name="gath", bufs=4))
        vpool = ctx.enter_context(tc.tile_pool(name="val", bufs=2))
        opool = ctx.enter_context(tc.tile_pool(name="outs", bufs=2))
        pspool = ctx.enter_context(
            tc.tile_pool(name="psum", bufs=1, space="PSUM"))
        trpool = ctx.enter_context(
            tc.tile_pool(name="trp", bufs=1, space="PSUM"))

        from concourse.masks import make_identity
        ident = cpool.tile([128, 128], bf)
        make_identity(nc, ident[:])
        # warm-up gather: preload the Q7 dma_gather library off the critical
        # path (values unused)
        wuidx = cpool.tile([128, 8], i16)
        nc.gpsimd.memset(wuidx[:], 0)
        wug = cpool.tile([128, 1, 1024], bf)
        for q in range(2):
            nc.gpsimd.dma_gather(
                out_ap=wug[:], in_ap=xq[0], idxs_ap=wuidx[:],
                num_idxs=128, num_idxs_reg=128, elem_size=1024,
                transpose=False, single_packet=False, queue_num=q)

        baset = cpool.tile([128, 8, 2 * K], f32)
        nc.sync.dma_start(baset[:], base_w[:, :, :])
        wtt = cpool.tile([128, 2 * K, C_OUT], bf)

        prep = {}
        for s in range(SPC):
            # ---- pipeline: (128, 8, 18) wrapped layout ----
            SH = [128, 8, 2 * K]
            offt = ppool.tile(SH, f32, tag="off")
            nc.sync.dma_start(offt[:], off_w[s])
            if s == 0:
                # on the scalar engine's HWDGE queue: keeps the 1.2MB weight
                # transfer off the sync queue that feeds the idx bounce
                nc.scalar.dma_start(wtt[:],
                                    wt[:, :, :].rearrange("m i o -> i m o"))
            py = tpool.tile(SH, f32, tag="py")
            nc.vector.tensor_tensor(py[:], offt[:], baset[:], AOT.add)
            fli = tpool.tile(SH, mybir.dt.int16, tag="fli")
            nc.vector.tensor_copy(fli[:], py[:])
            cf = tpool.tile(SH, f32, tag="cf")
            nc.vector.tensor_copy(cf[:], fli[:])
            gg = tpool.tile(SH, f32, tag="gg")
            nc.vector.tensor_tensor(gg[:], cf[:], py[:], AOT.is_gt)
            fl = tpool.tile(SH, f32, tag="fl")
            nc.vector.tensor_tensor(fl[:], cf[:], gg[:], AOT.subtract)
            flc = tpool.tile(SH, f32, tag="flc")
            nc.vector.tensor_scalar(flc[:], fl[:], 0.0, 31.0, AOT.max,
                                    AOT.min)
            t2 = tpool.tile([128, 8, K], f32, tag="t2")
            nc.vector.tensor_scalar(t2[:], flc[:, :, 0::2], float(W), None,
                                    AOT.mult)
            idxp = ppool.tile([128, 8, K], i16, tag="idxp")
            nc.vector.tensor_tensor(idxp[:], t2[:], flc[:, :, 1::2], AOT.add)
            frac = tpool.tile(SH, f32, tag="frac")
            nc.vector.tensor_tensor(frac[:], py[:], fl[:], AOT.subtract)
            a = tpool.tile(SH, f32, tag="a")           # [fl >= 0]
            nc.vector.tensor_scalar(a[:], fl[:], 0.0, None, AOT.is_ge)
            vb = tpool.tile(SH, f32, tag="vb")
            nc.vector.tensor_scalar(vb[:], fl[:], 31.0, None, AOT.is_le)
            v0 = tpool.tile(SH, f32, tag="v0")         # fl in [0,31]
            nc.vector.tensor_tensor(v0[:], a[:], vb[:], AOT.mult)
            va = tpool.tile(SH, f32, tag="va")
            nc.vector.tensor_scalar(va[:], fl[:], -1.0, None, AOT.is_ge)
            nc.vector.tensor_scalar(vb[:], fl[:], 30.0, None, AOT.is_le)
            v1 = tpool.tile(SH, f32, tag="v1")         # fl+1 in [0,31]
            nc.vector.tensor_tensor(v1[:], va[:], vb[:], AOT.mult)
            om = tpool.tile(SH, f32, tag="om")         # 1 - frac
            nc.vector.tensor_scalar(om[:], frac[:], -1.0, 1.0, AOT.mult,
                                    AOT.add)
            w0 = tpool.tile(SH, f32, tag="w0")         # lo-corner weight
            nc.vector.tensor_tensor(w0[:], om[:], v0[:], AOT.mult)
            w1 = tpool.tile(SH, f32, tag="w1")         # hi-corner weight
            nc.vector.tensor_tensor(w1[:], frac[:], v1[:], AOT.mult)
            # OOB remap onto clipped entry: wP0 = a*w0 + (1-a)*w1, wP1 = a*w1
            na = tpool.tile(SH, f32, tag="na")
            nc.vector.tensor_scalar(na[:], a[:], -1.0, 1.0, AOT.mult, AOT.add)
            t0 = tpool.tile(SH, f32, tag="t0")
            nc.vector.tensor_tensor(t0[:], a[:], w0[:], AOT.mult)
            t1 = tpool.tile(SH, f32, tag="t1")
            nc.vector.tensor_tensor(t1[:], na[:], w1[:], AOT.mult)
            wp0 = tpool.tile(SH, f32, tag="wp0")
            nc.vector.tensor_tensor(wp0[:], t0[:], t1[:], AOT.add)
            wp1 = tpool.tile(SH, f32, tag="wp1")
            nc.vector.tensor_tensor(wp1[:], a[:], w1[:], AOT.mult)

            # corner order in gathered entry: (y0,x0) (y1,x0) (y0,x1) (y1,x1)
            wplane = ppool.tile([128, 8, K, 4], f32, tag="wplane")
            wy0, wx0 = wp0[:, :, 0::2], wp0[:, :, 1::2]
            wy1, wx1 = wp1[:, :, 0::2], wp1[:, :, 1::2]
            nc.vector.tensor_tensor(wplane[:, :, :, 0], wy0, wx0, AOT.mult)
            nc.vector.tensor_tensor(wplane[:, :, :, 1], wy1, wx0, AOT.mult)
            nc.vector.tensor_tensor(wplane[:, :, :, 2], wy0, wx1, AOT.mult)
            nc.vector.tensor_tensor(wplane[:, :, :, 3], wy1, wx1, AOT.mult)

            # ---- bounce idx through DRAM into gather-wrapped layout ----
            ist = istage[s]  # (HW, K): addr = hw*K + k
            nc.sync.dma_start(
                bass.AP(ist.tensor, ist.offset,
                        [[K, 128], [128 * K, 8], [1, K]]),
                idxp[:])
            idxwt = ppool.tile([16, HW // 16, K], i16, tag="idxwt")
            nc.sync.dma_start(
                idxwt[:],
                bass.AP(ist.tensor, ist.offset,
                        [[K, 16], [16 * K, HW // 16], [1, K]]))
            idxw16 = ppool.tile([16, K, HW // 16], i16, tag="idxw16")
            nc.scalar.copy(
                idxw16[:],
                bass.AP(idxwt.tensor, idxwt.offset,
                        [[idxwt.ap[0][0], 16], [1, K], [K, HW // 16]]))
            idxw = ppool.tile([128, K, HW // 16], i16, tag="idxw")
            for grp in range(8):
                eng = nc.sync if grp % 2 == 0 else nc.scalar
                eng.dma_start(idxw[grp * 16:(grp + 1) * 16], idxw16[:])

            prep[s] = (idxw, wplane)

        for s in range(SPC):
            idxw, wplane = prep[s]
            ps = {}
            for oc in range(2):
                for hwin in range(2):
                    pst = pspool.tile([128, 512], f32, tag=f"ps{oc}{hwin}")
                    ps[(oc, hwin)] = pst

            for k in range(K):
                g2 = gpool.tile([128, 8, 1024], bf, tag="g2")
                for half in range(2):
                    nc.gpsimd.dma_gather(
                        out_ap=g2[:, half * 4:(half + 1) * 4, :],
                        in_ap=xq[s],
                        idxs_ap=idxw[:, k, half * 32:(half + 1) * 32],
                        num_idxs=HW // 2,
                        num_idxs_reg=HW // 2,
                        elem_size=1024,
                        transpose=False,
                        single_packet=True,
                        queue_num=half,
                    )
                # diag blocks: diagblk[p, px, b, q] = ident[p, q] * w[p, b, px]
                diagblk = vpool.tile([128, 4, 8, 128], bf, tag="diag")
                for px in range(4):
                    dv = diagblk[:, px, :, :]
                    wsl = wplane[:, :, k, px]
                    nc.vector.tensor_tensor(
                        dv,
                        bass.AP(ident.tensor, ident.offset,
                                [ident.ap[0], [0, 8], [1, 128]]),
                        bass.AP(wsl.tensor, wsl.offset,
                                [wsl.ap[0], [4 * K, 8], [0, 128]]),
                        AOT.mult)
                ptr = {}
                for cc in range(2):
                    pt = trpool.tile([128, 8, 128], f32, tag=f"tr{cc}")
                    ptr[cc] = pt
                # weighted transposes: ptr[cc][:, b] = sum_px g^T diag(w)
                for b in range(8):
                    for cc in range(2):
                        for px in range(4):
                            nc.tensor.matmul(
                                ptr[cc][:, b, :],
                                lhsT=g2[:, b, px * 256 + cc * 128:px * 256 + (cc * 128) + 128],
                                rhs=diagblk[:, px, b, :],
                                start=(px == 0),
                                stop=(px == 3),
                            )
                for cc in range(2):
                    valt = vpool.tile([128, HW], bf, tag=f"valt{cc}")
                    # evacuate the two PSUM transpose tiles on different
                    # engines so they run concurrently and free the banks
                    # for k+1's transposes sooner
                    src = ptr[cc][:].rearrange("p a b -> p (a b)")
                    if cc == 0:
                        nc.scalar.copy(valt[:], src)
                    else:
                        nc.vector.tensor_copy(valt[:], src)
                    for oc in range(2):
                        for hwin in range(2):
                            nc.tensor.matmul(
                                ps[(oc, hwin)][:],
                                lhsT=wtt[:, cc * K + k,
                                         oc * 128:(oc + 1) * 128],
                                rhs=valt[:, hwin * 512:(hwin + 1) * 512],
                                start=(k == 0 and cc == 0),
                                stop=(k == K - 1 and cc == 1),
                            )

            for oc in range(2):
                ot = opool.tile([128, HW], bf, tag="ot")
                for hwin in range(2):
                    nc.scalar.copy(ot[:, hwin * 512:(hwin + 1) * 512],
                                   ps[(oc, hwin)][:])
                nc.sync.dma_start(out_d[s][oc * 128:(oc + 1) * 128, :], ot[:])

    nc.compile()
    return nc


def get_nc():
    if "nc" not in _cache:
        _cache["nc"] = _build()
    return _cache["nc"]


def prep_core_inputs(x, offset, weight, core):
    """Host-side shard + layout for one core."""
    s0 = core * SPC
    xqs = np.zeros((SPC, NQROW, 1024), dtype=_BF16)
    offw = np.empty((SPC, 128, 8, 2 * K), dtype=np.float32)
    for i, s in enumerate(range(s0, s0 + SPC)):
        xt = x[s].reshape(C_IN, HW).T.astype(_BF16)         # (1024, 256)
        # entry q=(y*32+x): [(y,x) (y+1,x) (y,x+1) (y+1,x+1)]
        tab = np.zeros((NQROW, 1024), dtype=_BF16)
        tab[:HW, 0:256] = xt
        tab[:HW - W, 256:512] = xt[W:]
        tab[:HW - 1, 512:768] = xt[1:]
        tab[:HW - W - 1, 768:1024] = xt[W + 1:]
        xqs[i] = tab
        offw[i] = offset[s].reshape(8, 128, 2 * K).transpose(1, 0, 2)
    return {"xq": xqs, "off_w": offw}


def make_base_w():
    hwv = (np.arange(8)[None, :] * 128 + np.arange(128)[:, None])  # (128,8)
    ky = np.arange(K) // 3 - 1
    kx = np.arange(K) % 3 - 1
    base = np.empty((128, 8, 2 * K), dtype=np.float32)
    base[:, :, 0::2] = (hwv // W)[:, :, None] + ky[None, None, :]
    base[:, :, 1::2] = (hwv % W)[:, :, None] + kx[None, None, :]
    return base


def make_wt(weight):
    wk = weight.astype(np.float32).reshape(C_OUT, C_IN, K)  # (O, C, K)
    wt = np.empty((2 * K, 128, C_OUT), dtype=_BF16)
    for cc in range(2):
        for k in range(K):
            wt[cc * K + k] = wk[:, cc * 128:(cc + 1) * 128, k].T
    return wt


def _ensure_device():
    import subprocess
    import sys as _sys
    probe = (
        "import jax, numpy as np; "
        "x = jax.device_put(np.ones((4,4), np.float32), jax.devices()[0]); "
        "print('probe:', float((x+1).sum()))"
    )
    reset = (
        "import ctypes, jax, time; jax.devices(); "
        "lib = ctypes.CDLL('/opt/axon/libaxon_pjrt.so'); "
        "lib.axon_reset.restype = ctypes.c_int64; "
        "print('rc', lib.axon_reset()); time.sleep(2)"
    )
    r = subprocess.run([_sys.executable, "-c", probe], capture_output=True,
                       text=True, timeout=300)
    if "probe: 32.0" in r.stdout:
        return
    for _ in range(3):
        subprocess.run([_sys.executable, "-c", reset], timeout=300)
        r = subprocess.run([_sys.executable, "-c", probe],
                           capture_output=True, text=True, timeout=300)
        if "probe: 32.0" in r.stdout:
            return


def kernel(x, offset, weight):
    from concourse.bass_utils import run_bass_kernel_spmd

    _ensure_device()

    x = np.asarray(x, dtype=np.float32)
    offset = np.asarray(offset, dtype=np.float32)
    weight = np.asarray(weight, dtype=np.float32)
    nc = get_nc()
    base = make_base_w()
    wt = make_wt(weight)
    in_maps = []
    for c in range(NCORES):
        m = prep_core_inputs(x, offset, weight, c)
        m["base_w"] = base
        m["wt"] = wt
        in_maps.append(m)
    res = run_bass_kernel_spmd(nc, in_maps, core_ids=list(range(NCORES)))
    out = np.empty((N, C_OUT, H, W), dtype=np.float32)
    for c in range(NCORES):
        o = np.asarray(res.results[c]["out"], dtype=np.float32)
        out[c * SPC:(c + 1) * SPC] = o.reshape(SPC, C_OUT, H, W)
    return out



# revision 33
# speedup vs baseline: 1.0050x; 1.0050x over previous
"""Deformable 3x3 conv (AdaptiveConv, N=16 C=256 H=W=32) on 8 trn2 cores. V3.

Data-parallel over batch: 2 samples per core.
  - gather table (4-corner bf16 pack per entry) built host-side; per-tap
    HBM dma_gather (2KB/tap) on SWDGE queues 0/1 (queue 2/3 idx reads
    are slower on the Q7 pairs; keep 2 queues).
  - bilinear lerp folded into the PE: per corner px, matmul
    g_px^T @ diag(w_px) accumulated in PSUM replaces the DVE lerp and
    the separate PE transposes. DVE only builds diag blocks.
  - index chain (8 DVE ops) ordered before the corner-weight chain so
    the DRAM idx bounce and the first gather launch early.
  - the two PSUM transpose-tile evacuations run on scalar + vector
    concurrently, halving the PE stall between k-steps.
  - output written bf16 (PSUM accumulation stays f32).
"""
from contextlib import ExitStack

import numpy as np

try:
    import ml_dtypes
    _BF16 = ml_dtypes.bfloat16
except ImportError:  # pragma: no cover
    _BF16 = None

N, C_IN, C_OUT, H, W = 16, 256, 256, 32, 32
K = 9
HW = H * W
NCORES = 8
SPC = N // NCORES
NQROW = 1056  # gather table rows (>= 1025, padded)

_cache = {}


def _build():
    import concourse.bass as bass
    import concourse.mybir as mybir
    import concourse.tile as tile
    from concourse import bacc

    bf = mybir.dt.bfloat16
    f32 = mybir.dt.float32
    i8 = mybir.dt.int8
    i16 = mybir.dt.int16
    AOT = mybir.AluOpType

    nc = bacc.Bacc("TRN2", target_bir_lowering=False, debug=False,
                   num_devices=NCORES, num_swdge_queues=2)

    xq = nc.declare_dram_parameter("xq", [SPC, NQROW, 1024], bf,
                                   isOutput=False)
    off_w = nc.declare_dram_parameter("off_w", [SPC, 128, 8, 2 * K], f32,
                                      isOutput=False)
    base_w = nc.declare_dram_parameter("base_w", [128, 8, 2 * K], f32,
                                       isOutput=False)
    wt = nc.declare_dram_parameter("wt", [2 * K, 128, C_OUT], bf,
                                   isOutput=False)
    out_d = nc.declare_dram_parameter("out", [SPC, C_OUT, HW], bf,
                                      isOutput=True)

    istage = nc.dram_tensor("istage", [SPC, HW, K], i16)

    with tile.TileContext(nc) as tc, ExitStack() as ctx:
        cpool = ctx.enter_context(tc.tile_pool(name="const", bufs=1))
        ppool = ctx.enter_context(tc.tile_pool(name="pipe", bufs=2))
        tpool = ctx.enter_context(tc.tile_pool(name="tmp", bufs=2))
        gpool = ctx.enter_context(tc.tile_pool(name="gath", bufs=4))
        vpool = ctx.enter_context(tc.tile_pool(name="val", bufs=2))
        opool = ctx.enter_context(tc.tile_pool(name="outs", bufs=2))
        pspool = ctx.enter_context(
            tc.tile_pool(name="psum", bufs=1, space="PSUM"))
        trpool = ctx.enter_context(
            tc.tile_pool(name="trp", bufs=1, space="PSUM"))

        from concourse.masks import make_identity
        ident = cpool.tile([128, 128], bf)
        make_identity(nc, ident[:])
        # warm-up gather: preload the Q7 dma_gather library off the critical
        # path (values unused)
        wuidx = cpool.tile([128, 8], i16)
        nc.gpsimd.memset(wuidx[:], 0)
        wug = cpool.tile([128, 1, 1024], bf)
        for q in range(2):
            nc.gpsimd.dma_gather(
                out_ap=wug[:], in_ap=xq[0], idxs_ap=wuidx[:],
                num_idxs=128, num_idxs_reg=128, elem_size=1024,
                transpose=False, single_packet=False, queue_num=q)

        baset = cpool.tile([128, 8, 2 * K], f32)
        nc.sync.dma_start(baset[:], base_w[:, :, :])
        wtt = cpool.tile([128, 2 * K, C_OUT], bf)

        prep = {}
        for s in range(SPC):
            # ---- pipeline: (128, 8, 18) wrapped layout ----
            SH = [128, 8, 2 * K]
            offt = ppool.tile(SH, f32, tag="off")
            nc.sync.dma_start(offt[:], off_w[s])
            if s == 0:
                nc.sync.dma_start(wtt[:],
                                  wt[:, :, :].rearrange("m i o -> i m o"))
            py = tpool.tile(SH, f32, tag="py")
            nc.vector.tensor_tensor(py[:], offt[:], baset[:], AOT.add)
            fli = tpool.tile(SH, mybir.dt.int16, tag="fli")
            nc.vector.tensor_copy(fli[:], py[:])
            cf = tpool.tile(SH, f32, tag="cf")
            nc.vector.tensor_copy(cf[:], fli[:])
            gg = tpool.tile(SH, f32, tag="gg")
            nc.vector.tensor_tensor(gg[:], cf[:], py[:], AOT.is_gt)
            fl = tpool.tile(SH, f32, tag="fl")
            nc.vector.tensor_tensor(fl[:], cf[:], gg[:], AOT.subtract)
            flc = tpool.tile(SH, f32, tag="flc")
            nc.vector.tensor_scalar(flc[:], fl[:], 0.0, 31.0, AOT.max,
                                    AOT.min)
            t2 = tpool.tile([128, 8, K], f32, tag="t2")
            nc.vector.tensor_scalar(t2[:], flc[:, :, 0::2], float(W), None,
                                    AOT.mult)
            idxp = ppool.tile([128, 8, K], i16, tag="idxp")
            nc.vector.tensor_tensor(idxp[:], t2[:], flc[:, :, 1::2], AOT.add)
            frac = tpool.tile(SH, f32, tag="frac")
            nc.vector.tensor_tensor(frac[:], py[:], fl[:], AOT.subtract)
            a = tpool.tile(SH, f32, tag="a")           # [fl >= 0]
            nc.vector.tensor_scalar(a[:], fl[:], 0.0, None, AOT.is_ge)
            vb = tpool.tile(SH, f32, tag="vb")
            nc.vector.tensor_scalar(vb[:], fl[:], 31.0, None, AOT.is_le)
            v0 = tpool.tile(SH, f32, tag="v0")         # fl in [0,31]
            nc.vector.tensor_tensor(v0[:], a[:], vb[:], AOT.mult)
            va = tpool.tile(SH, f32, tag="va")
            nc.vector.tensor_scalar(va[:], fl[:], -1.0, None, AOT.is_ge)
            nc.vector.tensor_scalar(vb[:], fl[:], 30.0, None, AOT.is_le)
            v1 = tpool.tile(SH, f32, tag="v1")         # fl+1 in [0,31]
            nc.vector.tensor_tensor(v1[:], va[:], vb[:], AOT.mult)
            om = tpool.tile(SH, f32, tag="om")         # 1 - frac
            nc.vector.tensor_scalar(om[:], frac[:], -1.0, 1.0, AOT.mult,
                                    AOT.add)
            w0 = tpool.tile(SH, f32, tag="w0")         # lo-corner weight
            nc.vector.tensor_tensor(w0[:], om[:], v0[:], AOT.mult)
            w1 = tpool.tile(SH, f32, tag="w1")         # hi-corner weight
            nc.vector.tensor_tensor(w1[:], frac[:], v1[:], AOT.mult)
            # OOB remap onto clipped entry: wP0 = a*w0 + (1-a)*w1, wP1 = a*w1
            na = tpool.tile(SH, f32, tag="na")
            nc.vector.tensor_scalar(na[:], a[:], -1.0, 1.0, AOT.mult, AOT.add)
            t0 = tpool.tile(SH, f32, tag="t0")
            nc.vector.tensor_tensor(t0[:], a[:], w0[:], AOT.mult)
            t1 = tpool.tile(SH, f32, tag="t1")
            nc.vector.tensor_tensor(t1[:], na[:], w1[:], AOT.mult)
            wp0 = tpool.tile(SH, f32, tag="wp0")
            nc.vector.tensor_tensor(wp0[:], t0[:], t1[:], AOT.add)
            wp1 = tpool.tile(SH, f32, tag="wp1")
            nc.vector.tensor_tensor(wp1[:], a[:], w1[:], AOT.mult)

            # corner order in gathered entry: (y0,x0) (y1,x0) (y0,x1) (y1,x1)
            wplane = ppool.tile([128, 8, K, 4], f32, tag="wplane")
            wy0, wx0 = wp0[:, :, 0::2], wp0[:, :, 1::2]
            wy1, wx1 = wp1[:, :, 0::2], wp1[:, :, 1::2]
            nc.vector.tensor_tensor(wplane[:, :, :, 0], wy0, wx0, AOT.mult)
            nc.vector.tensor_tensor(wplane[:, :, :, 1], wy1, wx0, AOT.mult)
            nc.vector.tensor_tensor(wplane[:, :, :, 2], wy0, wx1, AOT.mult)
            nc.vector.tensor_tensor(wplane[:, :, :, 3], wy1, wx1, AOT.mult)

            # ---- bounce idx through DRAM into gather-wrapped layout ----
            ist = istage[s]  # (HW, K): addr = hw*K + k
            nc.sync.dma_start(
                bass.AP(ist.tensor, ist.offset,
                        [[K, 128], [128 * K, 8], [1, K]]),
                idxp[:])
            idxwt = ppool.tile([16, HW // 16, K], i16, tag="idxwt")
            nc.sync.dma_start(
                idxwt[:],
                bass.AP(ist.tensor, ist.offset,
                        [[K, 16], [16 * K, HW // 16], [1, K]]))
            idxw16 = ppool.tile([16, K, HW // 16], i16, tag="idxw16")
            nc.scalar.copy(
                idxw16[:],
                bass.AP(idxwt.tensor, idxwt.offset,
                        [[idxwt.ap[0][0], 16], [1, K], [K, HW // 16]]))
            idxw = ppool.tile([128, K, HW // 16], i16, tag="idxw")
            for grp in range(8):
                eng = nc.sync if grp % 2 == 0 else nc.scalar
                eng.dma_start(idxw[grp * 16:(grp + 1) * 16], idxw16[:])

            prep[s] = (idxw, wplane)

        for s in range(SPC):
            idxw, wplane = prep[s]
            ps = {}
            for oc in range(2):
                for hwin in range(2):
                    pst = pspool.tile([128, 512], f32, tag=f"ps{oc}{hwin}")
                    ps[(oc, hwin)] = pst

            for k in range(K):
                g2 = gpool.tile([128, 8, 1024], bf, tag="g2")
                for half in range(2):
                    nc.gpsimd.dma_gather(
                        out_ap=g2[:, half * 4:(half + 1) * 4, :],
                        in_ap=xq[s],
                        idxs_ap=idxw[:, k, half * 32:(half + 1) * 32],
                        num_idxs=HW // 2,
                        num_idxs_reg=HW // 2,
                        elem_size=1024,
                        transpose=False,
                        single_packet=True,
                        queue_num=half,
                    )
                # diag blocks: diagblk[p, px, b, q] = ident[p, q] * w[p, b, px]
                diagblk = vpool.tile([128, 4, 8, 128], bf, tag="diag")
                for px in range(4):
                    dv = diagblk[:, px, :, :]
                    wsl = wplane[:, :, k, px]
                    nc.vector.tensor_tensor(
                        dv,
                        bass.AP(ident.tensor, ident.offset,
                                [ident.ap[0], [0, 8], [1, 128]]),
                        bass.AP(wsl.tensor, wsl.offset,
                                [wsl.ap[0], [4 * K, 8], [0, 128]]),
                        AOT.mult)
                ptr = {}
                for cc in range(2):
                    pt = trpool.tile([128, 8, 128], f32, tag=f"tr{cc}")
                    ptr[cc] = pt
                # weighted transposes: ptr[cc][:, b] = sum_px g^T diag(w)
                for b in range(8):
                    for cc in range(2):
                        for px in range(4):
                            nc.tensor.matmul(
                                ptr[cc][:, b, :],
                                lhsT=g2[:, b, px * 256 + cc * 128:px * 256 + (cc * 128) + 128],
                                rhs=diagblk[:, px, b, :],
                                start=(px == 0),
                                stop=(px == 3),
                            )
                for cc in range(2):
                    valt = vpool.tile([128, HW], bf, tag=f"valt{cc}")
                    # evacuate the two PSUM transpose tiles on different
                    # engines so they run concurrently and free the banks
                    # for k+1's transposes sooner
                    src = ptr[cc][:].rearrange("p a b -> p (a b)")
                    if cc == 0:
                        nc.scalar.copy(valt[:], src)
                    else:
                        nc.vector.tensor_copy(valt[:], src)
                    for oc in range(2):
                        for hwin in range(2):
                            nc.tensor.matmul(
                                ps[(oc, hwin)][:],
                                lhsT=wtt[:, cc * K + k,
                                         oc * 128:(oc + 1) * 128],
                                rhs=valt[:, hwin * 512:(hwin + 1) * 512],
                                start=(k == 0 and cc == 0),
                                stop=(k == K - 1 and cc == 1),
                            )

            for oc in range(2):
                ot = opool.tile([128, HW], bf, tag="ot")
                for hwin in range(2):
                    nc.scalar.copy(ot[:, hwin * 512:(hwin + 1) * 512],
                                   ps[(oc, hwin)][:])
                nc.sync.dma_start(out_d[s][oc * 128:(oc + 1) * 128, :], ot[:])

    nc.compile()
    return nc


def get_nc():
    if "nc" not in _cache:
        _cache["nc"] = _build()
    return _cache["nc"]


def prep_core_inputs(x, offset, weight, core):
    """Host-side shard + layout for one core."""
    s0 = core * SPC
    xqs = np.zeros((SPC, NQROW, 1024), dtype=_BF16)
    offw = np.empty((SPC, 128, 8, 2 * K), dtype=np.float32)
    for i, s in enumerate(range(s0, s0 + SPC)):
        xt = x[s].reshape(C_IN, HW).T.astype(_BF16)         # (1024, 256)
        # entry q=(y*32+x): [(y,x) (y+1,x) (y,x+1) (y+1,x+1)]
        tab = np.zeros((NQROW, 1024), dtype=_BF16)
        tab[:HW, 0:256] = xt
        tab[:HW - W, 256:512] = xt[W:]
        tab[:HW - 1, 512:768] = xt[1:]
        tab[:HW - W - 1, 768:1024] = xt[W + 1:]
        xqs[i] = tab
        offw[i] = offset[s].reshape(8, 128, 2 * K).transpose(1, 0, 2)
    return {"xq": xqs, "off_w": offw}


def make_base_w():
    hwv = (np.arange(8)[None, :] * 128 + np.arange(128)[:, None])  # (128,8)
    ky = np.arange(K) // 3 - 1
    kx = np.arange(K) % 3 - 1
    base = np.empty((128, 8, 2 * K), dtype=np.float32)
    base[:, :, 0::2] = (hwv // W)[:, :, None] + ky[None, None, :]
    base[:, :, 1::2] = (hwv % W)[:, :, None] + kx[None, None, :]
    return base


def make_wt(weight):
    wk = weight.astype(np.float32).reshape(C_OUT, C_IN, K)  # (O, C, K)
    wt = np.empty((2 * K, 128, C_OUT), dtype=_BF16)
    for cc in range(2):
        for k in range(K):
            wt[cc * K + k] = wk[:, cc * 128:(cc + 1) * 128, k].T
    return wt


def _ensure_device():
    import subprocess
    import sys as _sys
    probe = (
        "import jax, numpy as np; "
        "x = jax.device_put(np.ones((4,4), np.float32), jax.devices()[0]); "
        "print('probe:', float((x+1).sum()))"
    )
    reset = (
        "import ctypes, jax, time; jax.devices(); "
        "lib = ctypes.CDLL('/opt/axon/libaxon_pjrt.so'); "
        "lib.axon_reset.restype = ctypes.c_int64; "
        "print('rc', lib.axon_reset()); time.sleep(2)"
    )
    r = subprocess.run([_sys.executable, "-c", probe], capture_output=True,
                       text=True, timeout=300)
    if "probe: 32.0" in r.stdout:
        return
    for _ in range(3):
        subprocess.run([_sys.executable, "-c", reset], timeout=300)
        r = subprocess.run([_sys.executable, "-c", probe],
                           capture_output=True, text=True, timeout=300)
        if "probe: 32.0" in r.stdout:
            return


def kernel(x, offset, weight):
    from concourse.bass_utils import run_bass_kernel_spmd

    _ensure_device()

    x = np.asarray(x, dtype=np.float32)
    offset = np.asarray(offset, dtype=np.float32)
    weight = np.asarray(weight, dtype=np.float32)
    nc = get_nc()
    base = make_base_w()
    wt = make_wt(weight)
    in_maps = []
    for c in range(NCORES):
        m = prep_core_inputs(x, offset, weight, c)
        m["base_w"] = base
        m["wt"] = wt
        in_maps.append(m)
    res = run_bass_kernel_spmd(nc, in_maps, core_ids=list(range(NCORES)))
    out = np.empty((N, C_OUT, H, W), dtype=np.float32)
    for c in range(NCORES):
        o = np.asarray(res.results[c]["out"], dtype=np.float32)
        out[c * SPC:(c + 1) * SPC] = o.reshape(SPC, C_OUT, H, W)
    return out

